# revision 11
# baseline (speedup 1.0000x reference)
"""Trainium2 Bass kernel for nn_DynamicQuantizedLinear.

Computes out = x @ dequant(W).T + bias + residual where
  x:[64,4096] f32, W_q:[11008,4096] int8, scale:[11008,32] f16 (group size 128),
  bias/residual:[11008] f16.

Strategy (column-parallel over out_features, 8 cores):
  - Host: dequantize W exactly to f32, then RE-quantize per output row to
    int8 with per-row scale t[o] (rel err ~6e-3 << 2e-2 tolerance). Ship
    int8 weights (1 byte/elem -> ~5.6MB/core, half the f16 traffic).
  - Device: slab 0 arrives as two 2-group pieces via the SWDGE (gpsimd)
    queue so casts start early and the SDMA engines pipeline two queues;
    remaining slabs stream on the sync HWDGE ring (weight DMAs must never
    be issued from the ACT engine: its descriptor generation queues
    behind cast ops and stalls the stream). DVE+ACT cast int8->f16 in
    rate-balanced column shares (DVE 2 elem/cyc @0.96GHz, ACT 1 @1.2GHz).
    PE runs two concurrent M=64 matmul streams on array column halves
    (tile_position (0,0)/(0,64)), each accumulating one half of the
    output features over all 32 K-groups. Per-row scale t applied in the
    PSUM->SBUF epilogue (tensor_tensor mult with a host-precomputed
    broadcast tile); bias/t enters PSUM via a K=1 ones matmul. Slab 7 is
    split 3+1 groups and the last group's matmuls retire the small PSUM
    chunk first so the epilogue/output overlaps the tail.
  - Output [128, 688] f16 (row blocks = feature halves); host reassembles.
"""

import numpy as np

OUT, IN, GS = 11008, 4096, 128
NG = IN // GS          # 32 groups
B = 64                 # batch rows
NCORES = 8
OPC = OUT // NCORES    # 1376 out features per core
HALF = OPC // 2        # 688 per PE column-half
NSLAB = 8              # quad-group int8 slabs
GPS = NG // NSLAB      # 4 groups per slab
CH = [(0, 512), (512, HALF - 512)]   # psum chunks within a half

_NC_CACHE = None


def _build():
    global _NC_CACHE
    if _NC_CACHE is not None:
        return _NC_CACHE

    import concourse.bacc as bacc
    import concourse.tile as tile
    import concourse.bass as bass
    import concourse.mybir as mybir

    f16 = mybir.dt.float16
    f32 = mybir.dt.float32
    i8 = mybir.dt.int8
    MULT = mybir.AluOpType.mult

    nc = bacc.Bacc(
        "TRN2", target_bir_lowering=False, debug=False, enable_asserts=False
    )
    # int8 weights, [k, o]-transposed, quad-group slabs: row = slab*128 + (k%128),
    # col = (group within slab)*1376 + o
    wt = nc.dram_tensor("wt", [NSLAB * 128, GPS * OPC], i8, kind="ExternalInput").ap()
    # x in [128, NG*B] group-major layout, then the per-row scale broadcast
    # tile (rows 0:64 = t[0:688], rows 64:128 = t[688:1376]) -- one DMA
    xtb = nc.dram_tensor(
        "xtb", [128, NG * B + HALF], f16, kind="ExternalInput"
    ).ap()
    # (bias+residual)/t, full width
    br = nc.dram_tensor("br", [1, OPC], f16, kind="ExternalInput").ap()
    out = nc.dram_tensor("out", [128, HALF], f16, kind="ExternalOutput").ap()

    # cast column split per slab, sized so DVE (2 elem/cyc @0.96GHz) and
    # ACT (1 elem/cyc @1.2GHz) finish together; GPSIMD casts measured 6x
    # slow AND degrade DVE's 2-port mode, so only DVE+ACT cast
    DVE_END = 3456

    with tile.TileContext(nc) as tc:
        with (
            tc.tile_pool(name="xp", bufs=1) as xpool,
            tc.tile_pool(name="w8", bufs=NSLAB + 1) as w8pool,
            tc.tile_pool(name="wf", bufs=3) as wfpool,
            tc.tile_pool(name="cp", bufs=1) as cpool,
            tc.tile_pool(name="op", bufs=1) as opool,
            tc.tile_pool(name="pp", bufs=1, space=bass.MemorySpace.PSUM) as pspool,
        ):
            xtbt = xpool.tile([128, NG * B + HALF], f16)
            xt = xtbt[:, : NG * B]
            tbt = xtbt[:, NG * B :]
            brt = cpool.tile([1, OPC], f16, tag="brt")
            nc.scalar.dma_start(xtbt[:], xtb[:])
            nc.scalar.dma_start(brt[:], br[:])

            ones = cpool.tile([1, B], f16, tag="ones")
            nc.gpsimd.memset(ones[:], 1.0)
            wsrc = cpool.tile([128, 512], f16, tag="wsrc")
            nc.gpsimd.memset(wsrc[:], 0.0)

            # slab 0 in two SWDGE pieces (early cast start + a second SDMA
            # queue while the sync ring streams s1+); s1-s6 whole on sync;
            # slab 7 split 3+1 groups for a fine-grained tail
            w8 = []
            mid0 = GPS * OPC // 2
            t8 = w8pool.tile([128, GPS * OPC], i8)
            nc.gpsimd.dma_start(t8[:, :mid0], wt[0:128, :mid0])
            nc.gpsimd.dma_start(t8[:, mid0:], wt[0:128, mid0:])
            w8.append(t8)
            for s in range(1, NSLAB - 1):
                t8 = w8pool.tile([128, GPS * OPC], i8)
                nc.sync.dma_start(t8[:], wt[s * 128 : (s + 1) * 128, :])
                w8.append(t8)
            t8 = w8pool.tile([128, GPS * OPC], i8)
            cut7 = 3 * OPC
            rows7 = slice((NSLAB - 1) * 128, NSLAB * 128)
            nc.sync.dma_start(t8[:, :cut7], wt[rows7, :cut7])
            nc.sync.dma_start(t8[:, cut7:], wt[rows7, cut7:])
            w8.append(t8)

            psA = pspool.tile([128, 512], f32, tag="psA", name="psA")
            psB = pspool.tile([128, HALF - 512], f32, tag="psB", name="psB")

            # HAM warm-up: full-array dummy matmuls while slab 0 streams in,
            # so the PE activity monitor unthrottles 1.2->2.4GHz early.
            warm_ps = pspool.tile([128, 512], f32, tag="warm", name="warm_ps")
            NWARM = 12
            for k in range(NWARM):
                nc.tensor.matmul(
                    warm_ps[:, :], wsrc[:, :128], wsrc[:, :],
                    start=(k == 0), stop=(k == NWARM - 1),
                )

            # bias/t into PSUM via K=1 ones matmul (per column-half, per chunk)
            for cg in range(2):
                rows = slice(64 * cg, 64 * (cg + 1))
                for (o0, n), ps in zip(CH, (psA, psB)):
                    nc.tensor.matmul(
                        ps[rows, :n],
                        ones[:, :],
                        brt[:, cg * HALF + o0 : cg * HALF + o0 + n],
                        start=True, stop=False,
                        # the two column-halves form disjoint element-range
                        # groups in the same bank; sim's check is bank-coarse
                        skip_group_check=True,
                    )

            # --- main pipeline: rate-balanced DVE|ACT casts, 4 matmuls/group
            def cast_slab(wfs, src, lo, hi, cut):
                cut = min(max(cut, lo), hi)
                if cut > lo:
                    nc.vector.tensor_copy(wfs[:, lo:cut], src[:, lo:cut])
                if hi > cut:
                    nc.scalar.copy(wfs[:, cut:hi], src[:, cut:hi])

            for s in range(NSLAB):
                wfs = wfpool.tile([128, GPS * OPC], f16)
                if s == 0:
                    # pieces matching slab 0's two SWDGE halves
                    cast_slab(wfs, w8[s], 0, mid0, 1728)
                    cast_slab(wfs, w8[s], mid0, GPS * OPC, mid0 + 1728)
                elif s < NSLAB - 1:
                    cast_slab(wfs, w8[s], 0, GPS * OPC, DVE_END)
                else:
                    # pieces matching the two tail DMA halves (3 + 1 groups)
                    cast_slab(wfs, w8[s], 0, cut7, 2592)
                    cast_slab(wfs, w8[s], cut7, GPS * OPC, cut7 + 864)
                for sub in range(GPS):
                    g = GPS * s + sub
                    xsl = xt[:, g * B : (g + 1) * B]
                    last = g == NG - 1
                    # retire the small B chunk first on the last group so the
                    # epilogue/output for B overlaps A's final matmuls
                    chunks = list(zip(CH, (psA, psB)))
                    if last:
                        chunks = chunks[::-1]
                    for (o0, n), ps in chunks:
                        for cg in range(2):
                            rows = slice(64 * cg, 64 * (cg + 1))
                            cols = slice(
                                sub * OPC + cg * HALF + o0,
                                sub * OPC + cg * HALF + o0 + n,
                            )
                            nc.tensor.matmul(
                                ps[rows, :n], xsl, wfs[:, cols],
                                start=False, stop=last,
                                skip_group_check=True,
                            )

            # --- epilogue: out = psum * t; B first, output DMAs on both rings
            osb = opool.tile([128, HALF], f16)
            epi = [(CH[1], psB, nc.scalar), (CH[0], psA, nc.sync)]
            for (o0, n), ps, ring in epi:
                nc.vector.tensor_tensor(
                    osb[:, o0 : o0 + n], ps[:, :n], tbt[:, o0 : o0 + n], MULT
                )
                ring.dma_start(out[:, o0 : o0 + n], osb[:, o0 : o0 + n])

    nc.compile()
    _NC_CACHE = nc
    return nc


def _prep_inputs(x, weight_q, scale, bias, weight_residual):
    """Host-side requant + shard + layout. Returns in_maps for 8 cores."""
    x = np.asarray(x, dtype=np.float32)
    weight_q = np.asarray(weight_q)
    scale = np.asarray(scale)
    bias = np.asarray(bias)
    weight_residual = np.asarray(weight_residual)
    # x [64, 4096] f32 -> [128 partitions(k within group), 32 groups, 64 batch] f16
    xgh = (
        x.reshape(B, NG, GS).transpose(2, 1, 0).astype(np.float16)
    ).reshape(128, NG * B)

    in_maps = []
    for c in range(NCORES):
        rows = slice(c * OPC, (c + 1) * OPC)
        wq_c = weight_q[rows]          # [1376, 4096] int8
        sc_c = scale[rows]             # [1376, 32] f16
        # exact f32 dequant, then per-row requant to int8
        wd = (
            wq_c.reshape(OPC, NG, GS).astype(np.float32)
            * sc_c.astype(np.float32)[:, :, None]
        ).reshape(OPC, IN)
        t = np.abs(wd).max(axis=1) / 127.0          # [1376] f32, > 0
        wq2 = np.clip(np.rint(wd / t[:, None]), -127, 127).astype(np.int8)
        # [4096, 1376] -> quad-group slab layout [8*128, 4*1376]
        wt_c = np.ascontiguousarray(
            wq2.T.reshape(NSLAB, GPS, 128, OPC)
            .transpose(0, 2, 1, 3)
            .reshape(NSLAB * 128, GPS * OPC)
        )
        tf = t.astype(np.float16)
        br_c = (
            (
                bias[rows].astype(np.float32)
                + weight_residual[rows].astype(np.float32)
            )
            / tf.astype(np.float32)
        ).astype(np.float16).reshape(1, OPC)
        tb_c = np.broadcast_to(
            tf.reshape(2, 1, HALF), (2, 64, HALF)
        ).reshape(128, HALF)
        xtb_c = np.ascontiguousarray(np.concatenate([xgh, tb_c], axis=1))
        in_maps.append(
            {"wt": wt_c, "xtb": xtb_c, "br": np.ascontiguousarray(br_c)}
        )
    return in_maps


def kernel(x, weight_q, scale, bias, weight_residual):
    from concourse.bass_utils import run_bass_kernel_spmd

    nc = _build()
    in_maps = _prep_inputs(x, weight_q, scale, bias, weight_residual)
    for _attempt in range(3):
        res = run_bass_kernel_spmd(nc, in_maps, core_ids=list(range(NCORES)))
        # [128, 688] per core: rows 0:64 = features 0:688, rows 64:128 = rest
        out = np.concatenate(
            [
                np.concatenate(
                    [res.results[c]["out"][:64], res.results[c]["out"][64:]], axis=1
                )
                for c in range(NCORES)
            ],
            axis=1,
        ).astype(np.float32)
        # guard against a rare transient on a freshly-loaded NEFF
        if np.isfinite(out).all():
            return out
    return out


# revision 12
# speedup vs baseline: 1.1605x; 1.1605x over previous
"""Trainium2 Bass kernel for nn_DynamicQuantizedLinear.

Computes out = x @ dequant(W).T + bias + residual where
  x:[64,4096] f32, W_q:[11008,4096] int8, scale:[11008,32] f16 (group size 128),
  bias/residual:[11008] f16.

Strategy (column-parallel over out_features, 8 cores):
  - Host: dequantize W exactly to f32, then RE-quantize per output row to
    int8 with per-row scale t[o] (rel err ~6e-3 << 2e-2 tolerance). Ship
    int8 weights (1 byte/elem -> ~5.6MB/core, half the f16 traffic).
  - Device: slab 0 arrives as two 2-group pieces via the SWDGE (gpsimd)
    queue so casts start early and the SDMA engines pipeline two queues;
    remaining slabs stream on the sync HWDGE ring (weight DMAs must never
    be issued from the ACT engine: its descriptor generation queues
    behind cast ops and stalls the stream). DVE+ACT cast int8->f16 in
    rate-balanced column shares (DVE 2 elem/cyc @0.96GHz, ACT 1 @1.2GHz).
    PE runs two concurrent M=64 matmul streams on array column halves
    (tile_position (0,0)/(0,64)), each accumulating one half of the
    output features over all 32 K-groups. Per-row scale t applied in the
    PSUM->SBUF epilogue (tensor_tensor mult with a host-precomputed
    broadcast tile); bias/t enters PSUM via a K=1 ones matmul. Slab 7 is
    split 3+1 groups and the last group's matmuls retire the small PSUM
    chunk first so the epilogue/output overlaps the tail.
  - Output [128, 688] f16 (row blocks = feature halves); host reassembles.
"""

import numpy as np

OUT, IN, GS = 11008, 4096, 128
NG = IN // GS          # 32 groups
B = 64                 # batch rows
NCORES = 8
OPC = OUT // NCORES    # 1376 out features per core
HALF = OPC // 2        # 688 per PE column-half
NSLAB = 8              # quad-group int8 slabs
GPS = NG // NSLAB      # 4 groups per slab
CH = [(0, 512), (512, HALF - 512)]   # psum chunks within a half

_NC_CACHE = None


def _build():
    global _NC_CACHE
    if _NC_CACHE is not None:
        return _NC_CACHE

    import concourse.bacc as bacc
    import concourse.tile as tile
    import concourse.bass as bass
    import concourse.mybir as mybir

    f16 = mybir.dt.float16
    f32 = mybir.dt.float32
    i8 = mybir.dt.int8
    MULT = mybir.AluOpType.mult

    nc = bacc.Bacc(
        "TRN2", target_bir_lowering=False, debug=False, enable_asserts=False
    )
    # int8 weights, [k, o]-transposed, quad-group slabs: row = slab*128 + (k%128),
    # col = (group within slab)*1376 + o
    wt = nc.dram_tensor("wt", [NSLAB * 128, GPS * OPC], i8, kind="ExternalInput").ap()
    # x in [128, NG*B] group-major layout, then the per-row scale broadcast
    # tile (rows 0:64 = t[0:688], rows 64:128 = t[688:1376]) -- one DMA
    xtb = nc.dram_tensor(
        "xtb", [128, NG * B + HALF], f16, kind="ExternalInput"
    ).ap()
    # (bias+residual)/t, full width
    br = nc.dram_tensor("br", [1, OPC], f16, kind="ExternalInput").ap()
    out = nc.dram_tensor("out", [128, HALF], f16, kind="ExternalOutput").ap()

    # cast column split per slab, sized so DVE (2 elem/cyc @0.96GHz) and
    # ACT (1 elem/cyc @1.2GHz) finish together; GPSIMD casts measured 6x
    # slow AND degrade DVE's 2-port mode, so only DVE+ACT cast
    DVE_END = 3456

    with tile.TileContext(nc) as tc:
        with (
            tc.tile_pool(name="xp", bufs=1) as xpool,
            tc.tile_pool(name="w8", bufs=NSLAB + 1) as w8pool,
            tc.tile_pool(name="wf", bufs=3) as wfpool,
            tc.tile_pool(name="cp", bufs=1) as cpool,
            tc.tile_pool(name="op", bufs=1) as opool,
            tc.tile_pool(name="pp", bufs=1, space=bass.MemorySpace.PSUM) as pspool,
        ):
            xtbt = xpool.tile([128, NG * B + HALF], f16)
            xt = xtbt[:, : NG * B]
            tbt = xtbt[:, NG * B :]
            brt = cpool.tile([1, OPC], f16, tag="brt")
            nc.scalar.dma_start(xtbt[:], xtb[:])
            nc.scalar.dma_start(brt[:], br[:])

            ones = cpool.tile([1, B], f16, tag="ones")
            nc.gpsimd.memset(ones[:], 1.0)
            wsrc = cpool.tile([128, 512], f16, tag="wsrc")
            nc.gpsimd.memset(wsrc[:], 0.0)

            # weight slabs 0-6 whole, slab 7 split 3+1 groups for a
            # fine-grained tail; all on the sync ring (second-ring and
            # SWDGE variants all stalled the stream's last transfers)
            w8 = []
            for s in range(0, NSLAB - 1):
                t8 = w8pool.tile([128, GPS * OPC], i8)
                nc.sync.dma_start(t8[:], wt[s * 128 : (s + 1) * 128, :])
                w8.append(t8)
            t8 = w8pool.tile([128, GPS * OPC], i8)
            cut7 = 3 * OPC
            rows7 = slice((NSLAB - 1) * 128, NSLAB * 128)
            nc.sync.dma_start(t8[:, :cut7], wt[rows7, :cut7])
            nc.sync.dma_start(t8[:, cut7:], wt[rows7, cut7:])
            w8.append(t8)

            psA = pspool.tile([128, 512], f32, tag="psA", name="psA")
            psB = pspool.tile([128, HALF - 512], f32, tag="psB", name="psB")

            # HAM warm-up: full-array dummy matmuls while slab 0 streams in,
            # so the PE activity monitor unthrottles 1.2->2.4GHz early.
            warm_ps = pspool.tile([128, 512], f32, tag="warm", name="warm_ps")
            NWARM = 12
            for k in range(NWARM):
                nc.tensor.matmul(
                    warm_ps[:, :], wsrc[:, :128], wsrc[:, :],
                    start=(k == 0), stop=(k == NWARM - 1),
                )

            # bias/t into PSUM via K=1 ones matmul (per column-half, per chunk)
            for cg in range(2):
                rows = slice(64 * cg, 64 * (cg + 1))
                for (o0, n), ps in zip(CH, (psA, psB)):
                    nc.tensor.matmul(
                        ps[rows, :n],
                        ones[:, :],
                        brt[:, cg * HALF + o0 : cg * HALF + o0 + n],
                        start=True, stop=False,
                        # the two column-halves form disjoint element-range
                        # groups in the same bank; sim's check is bank-coarse
                        skip_group_check=True,
                    )

            # --- main pipeline: rate-balanced DVE|ACT casts, 4 matmuls/group
            def cast_slab(wfs, src, lo, hi, cut):
                cut = min(max(cut, lo), hi)
                if cut > lo:
                    nc.vector.tensor_copy(wfs[:, lo:cut], src[:, lo:cut])
                if hi > cut:
                    nc.scalar.copy(wfs[:, cut:hi], src[:, cut:hi])

            for s in range(NSLAB):
                wfs = wfpool.tile([128, GPS * OPC], f16)
                if s < NSLAB - 1:
                    cast_slab(wfs, w8[s], 0, GPS * OPC, DVE_END)
                else:
                    # pieces matching the two tail DMA halves (3 + 1 groups)
                    cast_slab(wfs, w8[s], 0, cut7, 2592)
                    cast_slab(wfs, w8[s], cut7, GPS * OPC, cut7 + 864)
                for sub in range(GPS):
                    g = GPS * s + sub
                    xsl = xt[:, g * B : (g + 1) * B]
                    last = g == NG - 1
                    # retire the small B chunk first on the last group so the
                    # epilogue/output for B overlaps A's final matmuls
                    chunks = list(zip(CH, (psA, psB)))
                    if last:
                        chunks = chunks[::-1]
                    for (o0, n), ps in chunks:
                        for cg in range(2):
                            rows = slice(64 * cg, 64 * (cg + 1))
                            cols = slice(
                                sub * OPC + cg * HALF + o0,
                                sub * OPC + cg * HALF + o0 + n,
                            )
                            nc.tensor.matmul(
                                ps[rows, :n], xsl, wfs[:, cols],
                                start=False, stop=last,
                                skip_group_check=True,
                            )

            # --- epilogue: out = psum * t; B first, output DMAs on both rings
            osb = opool.tile([128, HALF], f16)
            epi = [(CH[1], psB, nc.scalar), (CH[0], psA, nc.sync)]
            for (o0, n), ps, ring in epi:
                nc.vector.tensor_tensor(
                    osb[:, o0 : o0 + n], ps[:, :n], tbt[:, o0 : o0 + n], MULT
                )
                ring.dma_start(out[:, o0 : o0 + n], osb[:, o0 : o0 + n])

    nc.compile()
    _NC_CACHE = nc
    return nc


def _prep_inputs(x, weight_q, scale, bias, weight_residual):
    """Host-side requant + shard + layout. Returns in_maps for 8 cores."""
    x = np.asarray(x, dtype=np.float32)
    weight_q = np.asarray(weight_q)
    scale = np.asarray(scale)
    bias = np.asarray(bias)
    weight_residual = np.asarray(weight_residual)
    # x [64, 4096] f32 -> [128 partitions(k within group), 32 groups, 64 batch] f16
    xgh = (
        x.reshape(B, NG, GS).transpose(2, 1, 0).astype(np.float16)
    ).reshape(128, NG * B)

    in_maps = []
    for c in range(NCORES):
        rows = slice(c * OPC, (c + 1) * OPC)
        wq_c = weight_q[rows]          # [1376, 4096] int8
        sc_c = scale[rows]             # [1376, 32] f16
        # exact f32 dequant, then per-row requant to int8
        wd = (
            wq_c.reshape(OPC, NG, GS).astype(np.float32)
            * sc_c.astype(np.float32)[:, :, None]
        ).reshape(OPC, IN)
        t = np.abs(wd).max(axis=1) / 127.0          # [1376] f32, > 0
        wq2 = np.clip(np.rint(wd / t[:, None]), -127, 127).astype(np.int8)
        # [4096, 1376] -> quad-group slab layout [8*128, 4*1376]
        wt_c = np.ascontiguousarray(
            wq2.T.reshape(NSLAB, GPS, 128, OPC)
            .transpose(0, 2, 1, 3)
            .reshape(NSLAB * 128, GPS * OPC)
        )
        tf = t.astype(np.float16)
        br_c = (
            (
                bias[rows].astype(np.float32)
                + weight_residual[rows].astype(np.float32)
            )
            / tf.astype(np.float32)
        ).astype(np.float16).reshape(1, OPC)
        tb_c = np.broadcast_to(
            tf.reshape(2, 1, HALF), (2, 64, HALF)
        ).reshape(128, HALF)
        xtb_c = np.ascontiguousarray(np.concatenate([xgh, tb_c], axis=1))
        in_maps.append(
            {"wt": wt_c, "xtb": xtb_c, "br": np.ascontiguousarray(br_c)}
        )
    return in_maps


def kernel(x, weight_q, scale, bias, weight_residual):
    from concourse.bass_utils import run_bass_kernel_spmd

    nc = _build()
    in_maps = _prep_inputs(x, weight_q, scale, bias, weight_residual)
    for _attempt in range(3):
        res = run_bass_kernel_spmd(nc, in_maps, core_ids=list(range(NCORES)))
        # [128, 688] per core: rows 0:64 = features 0:688, rows 64:128 = rest
        out = np.concatenate(
            [
                np.concatenate(
                    [res.results[c]["out"][:64], res.results[c]["out"][64:]], axis=1
                )
                for c in range(NCORES)
            ],
            axis=1,
        ).astype(np.float32)
        # guard against a rare transient on a freshly-loaded NEFF
        if np.isfinite(out).all():
            return out
    return out
